# revision 24
# baseline (speedup 1.0000x reference)
"""Trainium2 Bass kernel for MultiHeadDoublyStochasticSelfAttention.

Problem: b=8, n=1024, f=768, h=12, d=64; 3-step Sinkhorn (eps=1, row/col/row)
on softmax-free exp scores, then attn @ v and output projection.

Sharding: one batch element per NeuronCore (8 cores). Weights replicated.

Math (per head), single exp pass, all in exp domain, no reciprocals
(hardware divide ops only — DVE RECIPROCAL costs 1.4us/call):
  E^T[j,i] = exp(k_j . q_i)            (d^-0.5 folded into Wq on host)
  r_i   = sum_j E^T[j,i]               (PE ones-matvec, j on partitions)
  c~_j  = sum_i E^T[j,i] / r_i         (DVE scalar_tensor_tensor with
                                        op1=divide vs broadcast r, fused
                                        free-axis accum)
  vs    = [v | N] / c~_j               (DVE tensor_scalar divide)
  A[d,i] = sum_j vs[j,d] E^T[j,i]      (PE attn@v, raw E^T)
  row 64 of A = N sum_j E^T[j,i]/c~_j  -> out^T[d,i] = A[d,i] / row64[i]
(The per-i row factor a1_i cancels in the A/row64 ratio, so attn@v can
use the raw E^T.)
Then out^T = Wo @ concat_heads(out^T_head) + bo, host transposes back.
"""

import sys

if "/opt/trn_rl_repo" not in sys.path:
    sys.path.insert(0, "/opt/trn_rl_repo")

from contextlib import ExitStack

import numpy as np

import concourse.bass as bass
import concourse.mybir as mybir
import concourse.tile as tile

B, N, F, H, D = 8, 1024, 768, 12, 64
PC = F // 128        # 6 f-chunks of 128
TC = N // 128        # 8 token chunks of 128
F32 = mybir.dt.float32
BF16 = mybir.dt.bfloat16
EXP = mybir.ActivationFunctionType.Exp
IDENT = mybir.ActivationFunctionType.Identity
MUL = mybir.AluOpType.mult
ADD = mybir.AluOpType.add
DIV = mybir.AluOpType.divide


def _split_multi_waits(bir_bytes):
    """This container's walrus accepts at most ONE sync wait per instruction
    ("Too many sync wait commands"). Tile's semaphore pass attaches several.
    Rewrite the BIR: spill all but the last wait of each instruction onto
    same-engine NoOps placed directly before it (engines are in-order, so
    semantics are identical)."""
    import json

    d = json.loads(bir_bytes)
    uid = 0
    for fn in d["functions"]:
        for blk in fn["blocks"]:
            out = []
            for ins in blk["instructions"]:
                si = ins.get("sync_info")
                waits = (si or {}).get("on_wait") or []
                if len(waits) > 1:
                    for w in waits[:-1]:
                        uid += 1
                        out.append({
                            "debug": ins.get("debug", 0),
                            "engine": ins["engine"],
                            "ins": [], "outs": [],
                            "name": f"{ins['name']}-w{uid}",
                            "opcode": "NoOp",
                            "sync_info": {"on_update": [], "on_wait": [w]},
                            "text_hint": "split_wait",
                        })
                    si["on_wait"] = [waits[-1]]
                out.append(ins)
            blk["instructions"] = out
    return json.dumps(d).encode()


def build():
    nc = bass.Bass()
    xT = nc.declare_dram_parameter("xT", [F, N], BF16, isOutput=False)
    wqT = nc.declare_dram_parameter("wqT", [F, F], BF16, isOutput=False)
    wkT = nc.declare_dram_parameter("wkT", [F, F], BF16, isOutput=False)
    wvT = nc.declare_dram_parameter("wvT", [F, F], BF16, isOutput=False)
    woT = nc.declare_dram_parameter("woT", [F, F], BF16, isOutput=False)
    bo = nc.declare_dram_parameter("bo", [F], F32, isOutput=False)
    outT = nc.declare_dram_parameter("outT", [F, N], F32, isOutput=True)
    rrow_d = nc.dram_tensor("rrow_d", [H, N], F32)
    grow_d = nc.dram_tensor("grow_d", [H, N], F32)    # 1/(N T) rows
    growt_d = nc.dram_tensor("growt_d", [H, N], F32)  # raw N T rows

    with tile.TileContext(nc) as tc, ExitStack() as ctx:
        perm = ctx.enter_context(tc.tile_pool(name="perm", bufs=1))
        qt = [perm.tile([128, N], BF16, name=f"qt{i}", tag=f"qt{i}") for i in range(PC)]
        kt = [perm.tile([128, N], BF16, name=f"kt{i}", tag=f"kt{i}") for i in range(PC)]
        # v augmented with a column of N (for the gamma row) per head
        vg = [perm.tile([128, H * (D + 1)], BF16, name=f"vg{i}", tag=f"vg{i}")
              for i in range(TC)]
        ofT = [perm.tile([128, N], BF16, name=f"ofT{i}", tag=f"ofT{i}")
               for i in range(PC)]
        wo_sb = [perm.tile([128, F], BF16, name=f"wo{i}", tag=f"wo{i}")
                 for i in range(PC)]
        bo_sb = perm.tile([128, PC], F32, name="bo_sb", tag="bo_sb")
        ones_sb = perm.tile([128, 1], BF16, name="ones_sb", tag="ones_sb")
        onef_sb = perm.tile([128, 1], F32, name="onef_sb", tag="onef_sb")
        nc.vector.memset(ones_sb, 1.0)
        nc.vector.memset(onef_sb, 1.0)
        nc.sync.dma_start(out=bo_sb, in_=bo[:].rearrange("(c p) -> p c", p=128))
        for i in range(PC):
            nc.sync.dma_start(out=wo_sb[i], in_=woT[i * 128:(i + 1) * 128, :])
        for t in range(TC):
            # fill with N; the v-projection copies below overwrite the value
            # columns, leaving each head's 65th column = N (gamma-row trick)
            nc.vector.memset(vg[t], float(N))

        # ---------------- Phase A: q^T, k^T, v projections ----------------
        with tc.tile_pool(name="pxt", bufs=1) as pxt, \
             tc.tile_pool(name="pw", bufs=3 * PC) as pw, \
             tc.tile_pool(name="ppsa", bufs=3, space="PSUM") as ppsa:
            xt = [pxt.tile([128, N], BF16, name=f"xt{i}", tag=f"xt{i}")
                  for i in range(PC)]
            for i in range(PC):
                nc.sync.dma_start(out=xt[i], in_=xT[i * 128:(i + 1) * 128, :])

            w_all = {}
            for wname, wdram in (("q", wqT), ("k", wkT), ("v", wvT)):
                ws = []
                for kc in range(PC):
                    w = pw.tile([128, F], BF16, name=f"w{wname}{kc}", tag="w")
                    nc.sync.dma_start(out=w, in_=wdram[kc * 128:(kc + 1) * 128, :])
                    ws.append(w)
                w_all[wname] = ws

            NH = N // 2
            for wname, dst in (("q", qt), ("k", kt)):
                w_sb = w_all[wname]
                for mc in range(PC):
                    ps = ppsa.tile([128, N], F32, name="ps_a", tag="psa")
                    for hf in range(2):
                        for kc in range(PC):
                            nc.tensor.matmul(
                                ps[:, hf * NH:(hf + 1) * NH],
                                (w_sb[kc][:, mc * 128:(mc + 1) * 128]),
                                (xt[kc][:, hf * NH:(hf + 1) * NH]),
                                start=(kc == 0), stop=(kc == PC - 1),
                            )
                    nc.scalar.copy(dst[mc], ps)

            wv_sb = w_all["v"]
            for t in range(TC):
                ps = ppsa.tile([128, N], F32, name="ps_v", tag="psa")
                for hf, fw in ((0, NH), (1, F - NH)):
                    for kc in range(PC):
                        nc.tensor.matmul(
                            ps[:, hf * NH:hf * NH + fw],
                            (xt[kc][:, t * 128:(t + 1) * 128]),
                            (wv_sb[kc][:, hf * NH:hf * NH + fw]),
                            start=(kc == 0), stop=(kc == PC - 1),
                        )
                src = ps[:, :F].rearrange("p (h e) -> p h e", e=D)
                dst3 = vg[t].rearrange("p (h e) -> p h e", e=D + 1)
                nc.vector.tensor_copy(dst3[:, :, 0:D], src)

        # ---------------- Phase B: per-head sinkhorn attention ----------------
        # Software-pipelined at head granularity. Window t runs the exp(S^T)
        # pass + row-sum matvec of head t on PE/ScalarE while head t-1's
        # weighted col-sums (DVE) and attn@v (PE) stream through, so all three
        # engines stay continuously busy (HAM stays at 2.4 GHz).
        pe0t = ctx.enter_context(tc.tile_pool(name="pe0t", bufs=26))
        pscr = ctx.enter_context(tc.tile_pool(name="pscr", bufs=2))
        pa1 = ctx.enter_context(tc.tile_pool(name="pa1", bufs=2))
        psml = ctx.enter_context(tc.tile_pool(name="psml", bufs=2))
        pvs = ctx.enter_context(tc.tile_pool(name="pvs", bufs=4))
        pps_s = ctx.enter_context(tc.tile_pool(name="pps_s", bufs=2, space="PSUM"))
        pps_av = ctx.enter_context(tc.tile_pool(name="pps_av", bufs=2, space="PSUM"))
        # pass-1 windows of consecutive heads never overlap, so one buffer
        # suffices for the row-sum accumulator
        pps_r = ctx.enter_context(tc.tile_pool(name="pps_r", bufs=1, space="PSUM"))

        MV_LAG = 2      # row-sum matvec lags the exp pass
        STT_LAG = 2     # col-sum STT chain offset within its window
        AV_LAG = 2      # attn@v lags its vs chunk
        NS = 8 + 2

        def qk(h):
            hc, off = divmod(h, 2)
            off *= D
            return qt[hc][off:off + D, :], kt[hc][off:off + D, :]

        # Per-window station layout: cols 0-7 = c~ accums of head hB (by
        # j-chunk), cols 8-15 = (c p)-bounced gamma row of head hG. One
        # batched reciprocal serves both.
        state = {"e": {}, "av": {}}
        for t in range(H + 3):
            hA = t if t < H else None                 # pass 1: S^T+exp+rowsum
            hB = t - 1 if 1 <= t <= H else None       # col sums (STT)
            hC = t - 2 if 2 <= t <= H + 1 else None   # vs + attn@v
            hG = t - 3 if 3 <= t <= H + 2 else None   # gamma divide

            station = state.pop("station_next", None)
            if station is None and (hB is not None or hG is not None):
                station = psml.tile([128, 16], F32, name="station", tag="station")
                nc.vector.memset(station, 1.0)
            if hA is not None:
                qA, kA = qk(hA)
                r1 = pps_r.tile([1, N], F32, name="r_ps", tag="r")
                eA = [None] * TC
                state["e"][hA] = eA
            if hB is not None:
                rbcB = state.pop("rbc")
                eB = state["e"][hB]
            if hC is not None:
                avC = pps_av.tile([D + 1, N], F32, name="av_ps", tag="av")
                state["av"][hC] = avC
                eC = state["e"][hC]
                invC = state.pop("inv")
                vsC = [None] * TC

            for s in range(NS):
                # pass 1: S^T scores + exp (PSUM bank per 512-wide half)
                if hA is not None and s < TC:
                    e_sb = pe0t.tile([128, N], BF16, name="e_sb", tag="E")
                    eA[s] = e_sb
                    for ih in range(2):
                        ps = pps_s.tile([128, N // 2], F32, name="ps_s", tag="ps")
                        nc.tensor.matmul(
                            ps,
                            kA[:, s * 128:(s + 1) * 128],
                            qA[:, ih * (N // 2):(ih + 1) * (N // 2)],
                            start=True, stop=True,
                        )
                        nc.scalar.activation(
                            e_sb[:, ih * (N // 2):(ih + 1) * (N // 2)], ps, EXP
                        )

                # pass 1: raw row-sum matvec r_i = sum_j E^T[j,i]
                jc = s - MV_LAG
                if hA is not None and 0 <= jc < TC:
                    for ih in range(2):
                        nc.tensor.matmul(
                            r1[:, ih * (N // 2):(ih + 1) * (N // 2)],
                            ones_sb,
                            (eA[jc][:, ih * (N // 2):(ih + 1) * (N // 2)]),
                            start=(jc == 0), stop=(jc == TC - 1),
                            skip_group_check=True,
                        )
                    if jc == TC - 1:
                        a1row = pa1.tile([1, N], F32, name="a1row", tag="a1row")
                        nc.vector.reciprocal(a1row, r1)
                        nc.sync.dma_start(out=rrow_d[hA:hA + 1, :], in_=a1row)
                        rsrc = rrow_d[hA:hA + 1, :]
                        rbcA = pa1.tile([128, N], F32, name="rbc", tag="rbc")
                        nc.sync.dma_start(
                            out=rbcA,
                            in_=bass.AP(tensor=rsrc.tensor, offset=rsrc.offset,
                                        ap=[[0, 128]] + list(rsrc.ap[1:])),
                        )
                        state["rbc"] = rbcA

                # col sums of head hB: c~_j = sum_i E^T[j,i]/r_i
                jc = s - STT_LAG
                if hB is not None and 0 <= jc < TC:
                    scr = pscr.tile([128, N], BF16, name="scr", tag="scr")
                    with nc.allow_low_precision(reason="bf16 scratch"):
                        nc.vector.scalar_tensor_tensor(
                            scr, eB[jc], 1.0, rbcB, MUL, MUL,
                            accum_out=station[:, jc:jc + 1],
                        )

                # attn@v of head hC: vs = [v|N]*binv then accumulate matmuls
                if hC is not None and s < TC:
                    vs = pvs.tile([128, D + 1], BF16, name="vs", tag="vs")
                    vsC[s] = vs
                    nc.vector.tensor_scalar_mul(
                        vs, vg[s][:, hC * (D + 1):(hC + 1) * (D + 1)],
                        invC[:, s:s + 1],
                    )
                jc = s - AV_LAG
                if hC is not None and 0 <= jc < TC:
                    for ih in range(2):
                        nc.tensor.matmul(
                            avC[:, ih * (N // 2):(ih + 1) * (N // 2)],
                            vsC[jc],
                            (eC[jc][:, ih * (N // 2):(ih + 1) * (N // 2)]),
                            start=(jc == 0), stop=(jc == TC - 1),
                            skip_group_check=True,
                        )

            # ---- window tail ----
            if station is not None:
                # one reciprocal: binv of hB (cols 0-7) + ginv of hG (8-15)
                inv = psml.tile([128, 16], F32, name="inv", tag="inv")
                nc.vector.reciprocal(inv, station)
                state["inv"] = inv

            if hG is not None:
                # gamma: bounce ginv cols to a [1,N] DRAM row, broadcast to
                # [64,N], multiply the head rows
                inv_g = state["inv"]
                nc.sync.dma_start(
                    out=grow_d[hG:hG + 1, :].rearrange("o (c p) -> (o p) c",
                                                       p=128),
                    in_=inv_g[:, 8:16],
                )
                gb_sb = psml.tile([D, N], F32, name="gb_sb", tag="gb")
                gsrc = grow_d[hG:hG + 1, :]
                nc.sync.dma_start(
                    out=gb_sb,
                    in_=bass.AP(tensor=gsrc.tensor, offset=gsrc.offset,
                                ap=[[0, D]] + list(gsrc.ap[1:])),
                )
                avG = state["av"].pop(hG)
                hcz, offz = divmod(hG, 2)
                offz *= D
                nc.vector.tensor_mul(
                    ofT[hcz][offz:offz + D, :], avG[0:D, :], gb_sb
                )
                del state["e"][hG]

            if hC is not None:
                # bounce the gamma row of hC into the NEXT window's station
                growrow = pa1.tile([1, N], F32, name="growrow", tag="growrow")
                nc.scalar.copy(growrow, avC[D:D + 1, :])
                nc.sync.dma_start(out=growt_d[hC:hC + 1, :], in_=growrow)
                station_next = psml.tile([128, 16], F32, name="station",
                                         tag="station")
                nc.vector.memset(station_next[:, 0:8], 1.0)
                nc.sync.dma_start(
                    out=station_next[:, 8:16],
                    in_=growt_d[hC:hC + 1, :].rearrange("o (c p) -> (o p) c",
                                                        p=128),
                )
                state["station_next"] = station_next

        # ---------------- Phase C: output projection + bias ----------------
        for mc in range(PC):
            o_sb = pscr.tile([128, N], F32, name="o_sb", tag="osb")
            for hf in range(2):
                ps = pps_s.tile([128, N // 2], F32, name="ps_o", tag="ps")
                for kc in range(PC):
                    nc.tensor.matmul(
                        ps,
                        (wo_sb[kc][:, mc * 128:(mc + 1) * 128]),
                        (ofT[kc][:, hf * (N // 2):(hf + 1) * (N // 2)]),
                        start=(kc == 0), stop=(kc == PC - 1),
                    )
                nc.scalar.activation(
                    o_sb[:, hf * (N // 2):(hf + 1) * (N // 2)], ps, IDENT,
                    bias=bo_sb[:, mc:mc + 1],
                )
            nc.sync.dma_start(out=outT[mc * 128:(mc + 1) * 128, :], in_=o_sb)

    orig_to_json = nc.to_json_bytes
    nc.to_json_bytes = lambda: _split_multi_waits(orig_to_json())
    return nc


_NC = None


def _get_nc():
    global _NC
    if _NC is None:
        _NC = build()
    return _NC


def make_in_maps(x, Wq, Wk, Wv, Wo, bo):
    import ml_dtypes

    bf16 = ml_dtypes.bfloat16
    scale = np.float32(D ** -0.5)
    wq_t = np.ascontiguousarray((np.asarray(Wq) * scale).T.astype(bf16))
    wk_t = np.ascontiguousarray(np.asarray(Wk).T.astype(bf16))
    wv_t = np.ascontiguousarray(np.asarray(Wv).T.astype(bf16))
    wo_t = np.ascontiguousarray(np.asarray(Wo).T.astype(bf16))
    bo_c = np.ascontiguousarray(np.asarray(bo).astype(np.float32))
    maps = []
    for c in range(B):
        maps.append({
            "xT": np.ascontiguousarray(np.asarray(x[c]).T.astype(bf16)),
            "wqT": wq_t, "wkT": wk_t, "wvT": wv_t, "woT": wo_t, "bo": bo_c,
        })
    return maps


def kernel(x, Wq, Wk, Wv, Wo, bo):
    from concourse.bass_utils import run_bass_kernel_spmd

    x = np.asarray(x)
    nc = _get_nc()
    in_maps = make_in_maps(x, Wq, Wk, Wv, Wo, bo)
    res = run_bass_kernel_spmd(nc, in_maps, core_ids=list(range(B)))
    out = np.stack([res.results[c]["outT"].T.astype(np.float32) for c in range(B)],
                   axis=0)
    return out
